# revision 1
# baseline (speedup 1.0000x reference)
"""Trainium2 Bass kernel for single-head causal attention (nn_DefaultAttention).

Reference computation (B=4, S=2048, E=1024, fp32):
    k = x @ Wk.T + bk ; q = x @ Wq.T + bq ; v = x @ Wv.T + bv
    sim[b,s,t] = k[b,s]·q[b,t] / sqrt(E), masked to t<=s
    out[b,s]   = softmax_t(sim[b,s,:]) @ v[b,:]
i.e. standard causal attention with Q-role=k, K-role=q, V-role=v.

Sharding: 8 cores = 4 batches x 2 interleaved 128-row query-block sets.
A batch has 16 blocks of 128 rows with causal need n = block+1 key-tiles;
core h=0 takes blocks with needs {1,4,5,8,9,12,13,16}, h=1 takes
{2,3,6,7,10,11,14,15} (both sum 68 — balanced). SPMD requires an
identical instruction stream on every core, so the causal structure is
made uniform via a host-side per-core column permutation of x^T plus
data-driven 0/1 masks: slot k=0..7 processes a uniform T=2k+2 key-tiles
(pairwise max of the two cores' needs, sum 72 vs the causal-exact 68)
against the query block at permuted position 2k (PERM_BLOCKS places each
block so all its needed keys precede its slot window [0, T)). Invalid
(t>s) positions are zeroed after exp by per-core mask tensors.

All tensors stream as bfloat16 (matmuls run at 1 cycle/row regardless of
moving size; PSUM accumulates fp32; measured end-to-end rel err ~6e-3 vs
the 2e-2 gate). v is kept SBUF-resident (no DRAM spill/reload) in an
augmented layout [d0..383 | ones | d384..767 | d768..1023] so the attn@v
matmul produces the softmax denominator for free in column 384: the
fused phase computes out[s, d] = P^T@v_aug with P tiles as stationary
[t,128s] slices and v_aug as the moving operand, then normalizes by the
broadcast reciprocal of the den column on the DVE and stores out in
natural [s, d] orientation (no host transpose).
"""

import numpy as np


def _ensure_concourse():
    try:
        import concourse  # noqa: F401
    except ImportError:
        import sys
        for p in ("/opt/trn_rl_repo", "/root/.axon_site/_ro/trn_rl_repo"):
            if p not in sys.path:
                sys.path.append(p)
        import concourse  # noqa: F401


E = 1024
S = 2048
B = 4
NCORES = 8
ET = E // 128    # 8 feature tiles
ST = S // 128    # 16 key tiles
SCALE = 1.0 / np.sqrt(np.float32(E))
SLOT_T = (2, 4, 6, 8, 10, 12, 14, 16)   # key-128-tiles per slot (uniform)
SRC = tuple(256 * k for k in range(8))   # permuted query-col base per slot
MBASE = (0, 2, 6, 12, 20, 30, 42, 56)    # flat mask index base per slot
NMASK = 72
SLOT_ORDER = (7, 6, 5, 4, 3, 2, 1, 0)    # processing order (biggest T first)
# 128-row-block permutations: slot k (T=2k+2) hosts the core's query block
# with causal need n_k (A: {1,4,5,8,9,12,13,16}, B: {2,3,6,7,10,11,14,15})
# at position 2k; every block's needed key tiles land within [0, T_k).
PERM_BLOCKS = {
    0: [0, 1, 3, 2, 4, 5, 7, 6, 8, 9, 11, 10, 12, 13, 15, 14],
    1: [1, 0, 2, 3, 5, 4, 6, 7, 9, 8, 10, 11, 13, 12, 14, 15],
}
AUG = 1025                     # v_aug free width: 384 d | ones | 384 d | 256 d
DEN = 384                      # ones column index in v_aug

_CACHE = {}


def _build_program():
    _ensure_concourse()
    from contextlib import ExitStack
    import concourse.tile as tile
    import concourse.bass as bass
    from concourse import bacc, mybir

    F32 = mybir.dt.float32
    BF16 = mybir.dt.bfloat16
    ts = bass.ts
    Exp = mybir.ActivationFunctionType.Exp
    Ident = mybir.ActivationFunctionType.Identity

    nc = bacc.Bacc("TRN2", target_bir_lowering=False, debug=False)

    xT = nc.dram_tensor("xT", [E, S], BF16, kind="ExternalInput").ap()
    wkT = nc.dram_tensor("wkT", [E, E], BF16, kind="ExternalInput").ap()
    wqT = nc.dram_tensor("wqT", [E, E], BF16, kind="ExternalInput").ap()
    wvT = nc.dram_tensor("wvT", [E, E], BF16, kind="ExternalInput").ap()
    bkp = nc.dram_tensor("bkp", [128, ET], F32, kind="ExternalInput").ap()
    bqp = nc.dram_tensor("bqp", [128, ET], F32, kind="ExternalInput").ap()
    bv = nc.dram_tensor("bv", [E], F32, kind="ExternalInput").ap()
    masks = nc.dram_tensor("masks", [NMASK, 128, 128], BF16, kind="ExternalInput").ap()
    out_sd = nc.dram_tensor("out_sd", [1024, E], F32, kind="ExternalOutput").ap()

    with tile.TileContext(nc) as tc, ExitStack() as top:
        # ---- persistent smalls -------------------------------------------
        smalls = top.enter_context(tc.tile_pool(name="smalls", bufs=1))
        bk_sb = smalls.tile([128, ET], F32)
        nc.sync.dma_start(out=bk_sb, in_=bkp)
        bq_sb = smalls.tile([128, ET], F32)
        nc.sync.dma_start(out=bq_sb, in_=bqp)
        bvb = smalls.tile([128, E], F32)
        bv_bcast = bass.AP(tensor=bv.tensor, offset=bv.offset,
                           ap=[[0, 128]] + list(bv.ap))

        # Warm the ACT function tables (Identity/Exp) up front so the
        # LoadActFuncSet DMA doesn't queue behind the bulk loads later.
        scratch = smalls.tile([1, 8], F32)
        nc.vector.memset(scratch, 0.0)
        nc.scalar.activation(scratch, scratch, Ident, bias=0.0, scale=1.0)
        nc.scalar.activation(scratch, scratch, Exp, scale=1.0)

        # v_aug persists to the end; ones column initialized once.
        vaug_pool = top.enter_context(tc.tile_pool(name="vaug", bufs=1))
        v_aug = vaug_pool.tile([128, ST, AUG], BF16)
        nc.vector.memset(v_aug[:, :, DEN:DEN + 1], 1.0)

        # x^T + all three weights live together until the end of phase K.
        big_ctx = tc.tile_pool(name="big", bufs=1)
        big = big_ctx.__enter__()
        xt = big.tile([128, ET, S], BF16)
        wv = big.tile([128, ET, E], BF16)
        wq = big.tile([128, ET, E], BF16)
        wk = big.tile([128, ET, E], BF16)

        def load_xt(cb):
            for e in range(ET):
                nc.sync.dma_start(
                    out=xt[:, e, ts(cb, 512)],
                    in_=xT.rearrange("(e p) s -> p e s", p=128)[:, e, ts(cb, 512)],
                )

        def load_w(dst, src, db):
            # Weights ride the Activation HWDGE queue, in parallel with the
            # x^T stream on the SP queue.
            for e in range(ET):
                nc.sync.dma_start(
                    out=dst[:, e, ts(db, 512)],
                    in_=src.rearrange("(e p) f -> p e f", p=128)[:, e, ts(db, 512)],
                )

        # Consumption-ordered load stream: phase V needs wv+xt first; wq/wk
        # land during V (the DMA queue is otherwise idle — no v spill now).
        load_w(wv, wvT, 0)
        load_xt(0)
        load_xt(1)
        nc.sync.dma_start(out=bvb, in_=bv_bcast)
        load_w(wv, wvT, 1)
        load_xt(2)
        load_xt(3)
        load_w(wq, wqT, 0)
        load_w(wq, wqT, 1)
        load_w(wk, wkT, 0)
        load_w(wk, wkT, 1)

        # ---- phase V: v = x @ Wv.T + bv, resident in SBUF as v_aug -------
        # aug col mapping: [0:384]=d0..383, [384]=ones, [385:769]=d384..767,
        # [769:1025]=d768..1023.
        proj_ctx = tc.tile_pool(name="pproj", bufs=6, space="PSUM")
        proj_pool = proj_ctx.__enter__()
        if True:
            for db in range(2):
                for tt in range(ST):
                    pv = proj_pool.tile([128, 512], F32, tag="proj")
                    for e in range(ET):
                        nc.tensor.matmul(
                            pv, xt[:, e, ts(tt, 128)], wv[:, e, ts(db, 512)],
                            start=(e == 0), stop=(e == ET - 1),
                        )
                    if db == 0:
                        nc.vector.tensor_add(
                            v_aug[:, tt, 0:384], pv[:, 0:384], bvb[:, 0:384])
                        nc.vector.tensor_add(
                            v_aug[:, tt, 385:513], pv[:, 384:512],
                            bvb[:, 384:512])
                    else:
                        nc.vector.tensor_add(
                            v_aug[:, tt, 513:897], pv[:, 0:384],
                            bvb[:, 512:896])
                        nc.vector.tensor_add(
                            v_aug[:, tt, 897:1025], pv[:, 384:512],
                            bvb[:, 896:1024])

        # ---- phase Q: qT = (x @ Wq.T + bq)^T  ([f, t], all 2048 t) -------
        qt_pool = top.enter_context(tc.tile_pool(name="qt", bufs=1, side="right"))
        qt = qt_pool.tile([128, ET, S], BF16)
        if True:
            for ft in range(ET):
                for sb4 in range(4):
                    pq = proj_pool.tile([128, 512], F32, tag="proj")
                    for e in range(ET):
                        nc.tensor.matmul(
                            pq, wq[:, e, ts(ft, 128)], xt[:, e, ts(sb4, 512)],
                            start=(e == 0), stop=(e == ET - 1),
                        )
                    nc.scalar.activation(qt[:, ft, ts(sb4, 512)], pq, Ident,
                                         bias=bq_sb[:, ft:ft + 1])

        # ---- phase K: kT for local queries, slot sg at cols [256*sg,..) --
        kt_pool = top.enter_context(tc.tile_pool(name="kt", bufs=1, side="right"))
        kt = kt_pool.tile([128, ET, 1024], BF16)
        if True:
            for ft in range(ET):
                for sg in SLOT_ORDER:
                    pk = proj_pool.tile([128, 512], F32, tag="proj")
                    for e in range(ET):
                        nc.tensor.matmul(
                            pk[:, 0:128], wk[:, e, ts(ft, 128)],
                            xt[:, e, SRC[sg]:SRC[sg] + 128],
                            start=(e == 0), stop=(e == ET - 1),
                        )
                    nc.scalar.activation(kt[:, ft, ts(sg, 128)], pk[:, 0:128],
                                         Ident, bias=bk_sb[:, ft:ft + 1])
        proj_ctx.__exit__(None, None, None)
        big_ctx.__exit__(None, None, None)

        # ---- fused scores + attn@v, interleaved per slot -----------------
        # Emission pattern: scores(s3), scores(s2), d(s3), scores(s1),
        # d(s2), scores(s0), d(s1), d(s0) - each d-loop's P tiles are ready
        # well before it issues; scores/d-loop PSUM pools coexist within the
        # 8-bank budget.
        p_pool = top.enter_context(tc.tile_pool(name="pP", bufs=1))
        p_tiles = {}

        with tc.tile_pool(name="mask", bufs=4) as mask_pool, \
             tc.tile_pool(name="osb", bufs=3) as osb_pool, \
             tc.tile_pool(name="denr", bufs=2) as denr_pool, \
             tc.tile_pool(name="ps", bufs=2, space="PSUM") as ps_pool, \
             tc.tile_pool(name="po", bufs=2, space="PSUM") as po_pool:

            def emit_scores(sg):
                T = SLOT_T[sg]
                for j in range(T):
                    ps = ps_pool.tile([128, 128], F32, tag="ps",
                                      name=f"ps_{sg}_{j}")
                    for e in range(ET):
                        nc.tensor.matmul(
                            ps, qt[:, e, ts(j, 128)], kt[:, e, ts(sg, 128)],
                            start=(e == 0), stop=(e == ET - 1),
                        )
                    P = p_pool.tile([128, 128], BF16, tag=f"P{j}",
                                    name=f"P_{sg}_{j}", bufs=2)
                    nc.scalar.activation(P, ps, Exp, scale=float(SCALE))
                    m = mask_pool.tile([128, 128], BF16, tag="mask",
                                       name=f"m_{sg}_{j}")
                    nc.sync.dma_start(out=m, in_=masks[MBASE[sg] + j, :, :])
                    nc.vector.tensor_mul(P, P, m)
                    p_tiles[(sg, j)] = P

            def emit_dloop(sg):
                T = SLOT_T[sg]
                last = sg == SLOT_ORDER[-1]
                rows = 128 * sg
                poA = po_pool.tile([128, 385], F32, tag="poA",
                                   name=f"poA_{sg}")
                poB = po_pool.tile([128, 384], F32, tag="poB",
                                   name=f"poB_{sg}")
                poC = po_pool.tile([128, 256], F32, tag="poC",
                                   name=f"poC_{sg}")
                for j in range(T):
                    nc.tensor.matmul(
                        poA, p_tiles[(sg, j)], v_aug[:, j, 0:385],
                        start=(j == 0), stop=(j == T - 1),
                    )
                for j in range(T):
                    nc.tensor.matmul(
                        poB, p_tiles[(sg, j)], v_aug[:, j, 385:769],
                        start=(j == 0), stop=(j == T - 1),
                    )
                for j in range(T):
                    nc.tensor.matmul(
                        poC, p_tiles[(sg, j)], v_aug[:, j, 769:1025],
                        start=(j == 0), stop=(j == T - 1),
                    )
                dr = denr_pool.tile([128, 1], F32, tag="dr", name=f"dr_{sg}")
                nc.vector.reciprocal(dr, poA[:, DEN:DEN + 1])
                drb = bass.AP(tensor=dr.tensor, offset=dr.offset,
                              ap=[list(dr.ap[0]), [0, 384]])
                osb = osb_pool.tile([128, E], F32, tag="osb", name=f"osb_{sg}")
                nc.vector.tensor_mul(osb[:, 0:384], poA[:, 0:384], drb)
                nc.vector.tensor_mul(osb[:, 384:768], poB, drb)
                nc.vector.tensor_mul(
                    osb[:, 768:1024], poC,
                    bass.AP(tensor=dr.tensor, offset=dr.offset,
                            ap=[list(dr.ap[0]), [0, 256]]))
                if last:
                    # Final slot: chunked store so the DMA overlaps the
                    # trailing DVE work instead of waiting for all of it.
                    nc.sync.dma_start(out=out_sd[rows:rows + 128, 0:384],
                                      in_=osb[:, 0:384])
                    nc.sync.dma_start(out=out_sd[rows:rows + 128, 384:768],
                                      in_=osb[:, 384:768])
                    nc.sync.dma_start(out=out_sd[rows:rows + 128, 768:1024],
                                      in_=osb[:, 768:1024])
                else:
                    nc.sync.dma_start(out=out_sd[rows:rows + 128, :], in_=osb)

            emit_scores(SLOT_ORDER[0])
            for idx, sg in enumerate(SLOT_ORDER):
                nxt = SLOT_ORDER[idx + 1] if idx + 1 < len(SLOT_ORDER) else None
                if nxt is not None:
                    emit_scores(nxt)
                emit_dloop(sg)

    nc.compile()
    return nc


def _get_program():
    if "nc" not in _CACHE:
        _CACHE["nc"] = _build_program()
    return _CACHE["nc"]


def _perm_indices(h):
    return np.concatenate(
        [np.arange(128 * b, 128 * (b + 1)) for b in PERM_BLOCKS[h]])


def _host_prep(x, Wk, bk, Wq, bq, Wv, bv):
    """Build per-core in_maps (bf16 streams)."""
    import ml_dtypes
    f32 = np.float32
    bf16 = ml_dtypes.bfloat16
    wkT = np.ascontiguousarray(Wk.T).astype(bf16)
    wqT = np.ascontiguousarray(Wq.T).astype(bf16)
    wvT = np.ascontiguousarray(Wv.T).astype(bf16)
    bkp = np.ascontiguousarray(bk.astype(f32).reshape(ET, 128).T)
    bqp = np.ascontiguousarray(bq.astype(f32).reshape(ET, 128).T)
    bvc = np.ascontiguousarray(bv.astype(f32))

    in_maps = []
    for c in range(NCORES):
        b, h = divmod(c, 2)
        perm = _perm_indices(h)
        xTb = np.ascontiguousarray(np.asarray(x[b]).T[:, perm]).astype(bf16)
        m = np.zeros((NMASK, 128, 128), f32)
        for sg in range(8):
            s_g = perm[SRC[sg]:SRC[sg] + 128]
            for j in range(SLOT_T[sg]):
                t_g = perm[128 * j:128 * (j + 1)]
                m[MBASE[sg] + j] = (t_g[:, None] <= s_g[None, :]).astype(f32)
        in_maps.append({
            "xT": xTb, "wkT": wkT, "wqT": wqT, "wvT": wvT,
            "bkp": bkp, "bqp": bqp, "bv": bvc, "masks": m.astype(bf16),
        })
    return in_maps


def _assemble(results):
    out = np.empty((B, S, E), np.float32)
    for c in range(NCORES):
        b, h = divmod(c, 2)
        perm = _perm_indices(h)
        osd = results[c]["out_sd"]  # [1024, E], rows = slot-local queries
        for sg in range(8):
            rows = perm[SRC[sg]:SRC[sg] + 128]
            out[b, rows, :] = osd[128 * sg:128 * (sg + 1), :]
    return out


def kernel(x, Wk, bk, Wq, bq, Wv, bv):
    _ensure_concourse()
    from concourse.bass_utils import run_bass_kernel_spmd
    nc = _get_program()
    in_maps = _host_prep(x, Wk, bk, Wq, bq, Wv, bv)
    res = run_bass_kernel_spmd(nc, in_maps, list(range(NCORES)))
    return _assemble(res.results)



# revision 5
# speedup vs baseline: 1.4771x; 1.4771x over previous
"""Trainium2 Bass kernel for single-head causal attention (nn_DefaultAttention).

Reference computation (B=4, S=2048, E=1024, fp32):
    k = x @ Wk.T + bk ; q = x @ Wq.T + bq ; v = x @ Wv.T + bv
    sim[b,s,t] = k[b,s]·q[b,t] / sqrt(E), masked to t<=s
    out[b,s]   = softmax_t(sim[b,s,:]) @ v[b,:]

Algebraic folding (single head => full-rank fold is exact):
    sim*sqrt(E) = k·q^T = x (Wk^T Wq) x^T + per-s + per-t + const terms.
    The per-s and const bias terms are constant along the softmax axis t
    and cancel; the per-t term (Wq^T bk)·x[t] folds into a per-feature
    bias on z := x M (M = Wk^T Wq, host-computed).  So
        scores[s,t] = (x[s] M + c) · x[t],   c = Wq^T bk
    and the q/k projections never happen on-device.  Likewise
        out = P @ v / den = (P @ x) @ Wv^T / den + bv
    so the v projection becomes a post-GEMM on the (normalized) P@x.

Per-core work drops from 2.5 projection-equivalents + attention to 1.0
(z for the local 1024 queries + final Wv GEMM) + attention.

Sharding: 8 cores = 4 batches x 2 interleaved 128-row query-block sets
(same scheme as before: slot k=0..7 processes T=2k+2 key tiles against
the query block at permuted position 2k; per-core masks zero invalid
t>s entries after exp).

Dataflow per core (all GEMM operands bf16, PSUM fp32):
    z^T[e',s]  = sum_e M32[e,e'] xTq[e,s] / 32 + c[e']   (ACT bias+scale)
    ps[t,s]    = sum_e xT[e,t] z^T[e,s]                  (per slot, t<T)
    P[t,s]     = exp(ps/sqrt(E)) * mask  (ACT+DVE)
    uT[e,s]    = sum_t xrow[t,e] P[t,s]  (+ den row via ones column)
    uT_norm    = uT * (1/den)            (DVE, partition-broadcast)
    out[s,f]   = sum_e uT_norm[e,s] wvT[e,f] (+bv)       (DVE bias add)
"""

import numpy as np


def _ensure_concourse():
    try:
        import concourse  # noqa: F401
    except ImportError:
        import sys
        for p in ("/opt/trn_rl_repo", "/root/.axon_site/_ro/trn_rl_repo"):
            if p not in sys.path:
                sys.path.append(p)
        import concourse  # noqa: F401


E = 1024
S = 2048
B = 4
NCORES = 8
ET = E // 128    # 8 feature tiles
ST = S // 128    # 16 key tiles
SCALE = 1.0 / np.sqrt(np.float32(E))
SLOT_T = (2, 4, 6, 8, 10, 12, 14, 16)   # key-128-tiles per slot (uniform)
SRC = tuple(256 * k for k in range(8))   # permuted query-col base per slot
MBASE = (0, 2, 6, 12, 20, 30, 42, 56)    # flat mask index base per slot
NMASK = 72
SLOT_ORDER = (7, 6, 5, 4, 3, 2, 1, 0)    # processing order (biggest T first)
# 128-row-block permutations: slot k (T=2k+2) hosts the core's query block
# with causal need n_k (A: {1,4,5,8,9,12,13,16}, B: {2,3,6,7,10,11,14,15})
# at position 2k; every block's needed key tiles land within [0, T_k).
PERM_BLOCKS = {
    0: [0, 1, 3, 2, 4, 5, 7, 6, 8, 9, 11, 10, 12, 13, 15, 14],
    1: [1, 0, 2, 3, 5, 4, 6, 7, 9, 8, 10, 11, 13, 12, 14, 15],
}
XA = 1025                      # xrow free width: 1024 features + ones col

_CACHE = {}


def _build_program():
    _ensure_concourse()
    from contextlib import ExitStack
    import concourse.tile as tile
    import concourse.bass as bass
    from concourse import bacc, mybir

    F32 = mybir.dt.float32
    BF16 = mybir.dt.bfloat16
    ts = bass.ts
    Exp = mybir.ActivationFunctionType.Exp
    Ident = mybir.ActivationFunctionType.Identity

    nc = bacc.Bacc("TRN2", target_bir_lowering=False, debug=False)

    xT = nc.dram_tensor("xT", [E, S], BF16, kind="ExternalInput").ap()
    xTq = nc.dram_tensor("xTq", [E, 1024], BF16, kind="ExternalInput").ap()
    xrow = nc.dram_tensor("xrow", [128, ST, XA], BF16, kind="ExternalInput").ap()
    mw = nc.dram_tensor("mw", [E, E], BF16, kind="ExternalInput").ap()
    wvT = nc.dram_tensor("wvT", [E, E], BF16, kind="ExternalInput").ap()
    zbp = nc.dram_tensor("zbp", [128, ET], F32, kind="ExternalInput").ap()
    bv = nc.dram_tensor("bv", [E], F32, kind="ExternalInput").ap()
    masks = nc.dram_tensor("masks", [NMASK, 128, 128], BF16, kind="ExternalInput").ap()
    out_sd = nc.dram_tensor("out_sd", [1024, E], F32, kind="ExternalOutput").ap()

    with tile.TileContext(nc) as tc, ExitStack() as top:
        # ---- persistent smalls -------------------------------------------
        smalls = top.enter_context(tc.tile_pool(name="smalls", bufs=1))
        zb_sb = smalls.tile([128, ET], F32)
        nc.sync.dma_start(out=zb_sb, in_=zbp)
        bvb = smalls.tile([128, E], F32)
        bv_bcast = bass.AP(tensor=bv.tensor, offset=bv.offset,
                           ap=[[0, 128]] + list(bv.ap))
        ones_mv = smalls.tile([128, 1], BF16)
        nc.vector.memset(ones_mv, 1.0)

        # Warm the ACT function tables (Identity/Exp) up front so the
        # LoadActFuncSet DMA doesn't queue behind the bulk loads later.
        scratch = smalls.tile([1, 8], F32)
        nc.vector.memset(scratch, 0.0)
        nc.scalar.activation(scratch, scratch, Ident, bias=0.0, scale=1.0)
        nc.scalar.activation(scratch, scratch, Exp, scale=1.0)

        # ---- big persistent operands -------------------------------------
        big = top.enter_context(tc.tile_pool(name="big", bufs=1))
        xt = big.tile([128, ET, S], BF16)       # x^T permuted (key side)
        xr = big.tile([128, ST, XA], BF16)      # x row-major + ones col
        wv = big.tile([128, ET, E], BF16)       # Wv^T as [e, f]
        zt = big.tile([128, ET, 1024], BF16)    # z^T (local queries)

        # xtq + mw only live through phase Z; placed in a scoped pool.
        zin_ctx = tc.tile_pool(name="zin", bufs=1, side="right")
        zin = zin_ctx.__enter__()
        xtq = zin.tile([128, ET, 1024], BF16)
        mwt = zin.tile([128, ET, E], BF16)

        def load_rearr(dst, src, db, width=512):
            for e in range(ET):
                nc.sync.dma_start(
                    out=dst[:, e, ts(db, width)],
                    in_=src.rearrange("(e p) s -> p e s", p=128)[:, e, ts(db, width)],
                )

        def load_xr(jlo, jhi):
            for j in range(jlo, jhi):
                nc.sync.dma_start(out=xr[:, j, :], in_=xrow[:, j, :])

        # Consumption order: phase Z needs mw+xtq first; xT / xrow / wvT
        # land during Z (DMA queues otherwise idle).
        load_rearr(mwt, mw, 0, 512)
        load_rearr(xtq, xTq, 0, 512)
        load_rearr(mwt, mw, 1, 512)
        load_rearr(xtq, xTq, 1, 512)
        nc.sync.dma_start(out=bvb, in_=bv_bcast)
        for cb in range(4):
            load_rearr(xt, xT, cb, 512)
        load_xr(0, ST)
        load_rearr(wv, wvT, 0, 512)
        load_rearr(wv, wvT, 1, 512)

        # ---- phase Z: z^T = (x M32)/32 + c, local queries ----------------
        proj_ctx = tc.tile_pool(name="pz", bufs=4, space="PSUM")
        proj_pool = proj_ctx.__enter__()
        for ep in range(ET):
            for sb in range(2):
                pz = proj_pool.tile([128, 512], F32, tag="pz")
                for e in range(ET):
                    nc.tensor.matmul(
                        pz, mwt[:, e, ts(ep, 128)], xtq[:, e, ts(sb, 512)],
                        start=(e == 0), stop=(e == ET - 1),
                    )
                nc.scalar.activation(zt[:, ep, ts(sb, 512)], pz, Ident,
                                     bias=zb_sb[:, ep:ep + 1],
                                     scale=float(1.0 / 32.0))
        proj_ctx.__exit__(None, None, None)
        zin_ctx.__exit__(None, None, None)

        # ---- fused scores -> P -> u^T -> final GEMM, per slot ------------
        # PSUM budget (8 banks): ps pairs 3 + pu 2 + pd 1 + po 2.
        p_pool = top.enter_context(tc.tile_pool(name="pP", bufs=1))
        p_tiles = {}

        with tc.tile_pool(name="mask", bufs=4) as mask_pool, \
             tc.tile_pool(name="osb", bufs=3) as osb_pool, \
             tc.tile_pool(name="ut", bufs=2) as ut_pool, \
             tc.tile_pool(name="denr", bufs=2) as denr_pool, \
             tc.tile_pool(name="ps", bufs=3, space="PSUM") as ps_pool, \
             tc.tile_pool(name="pu", bufs=1, space="PSUM") as pu_pool, \
             tc.tile_pool(name="pd", bufs=1, space="PSUM") as pd_pool, \
             tc.tile_pool(name="po", bufs=2, space="PSUM") as po_pool:

            ut_tiles = {}

            def emit_scores(sg):
                # j-pairs share one PSUM bank; chains are emitted
                # sequentially so the pending-zero region flip is safe.
                T = SLOT_T[sg]
                for jp in range(T // 2):
                    ps = ps_pool.tile([128, 2, 128], F32, tag="ps",
                                      name=f"ps_{sg}_{jp}")
                    for jj in range(2):
                        for e in range(ET):
                            nc.tensor.matmul(
                                ps[:, jj, :], xt[:, e, ts(2 * jp + jj, 128)],
                                zt[:, e, ts(sg, 128)],
                                start=(e == 0), stop=(e == ET - 1),
                            )
                    P = p_pool.tile([128, 2, 128], BF16, tag=f"P{jp}",
                                    name=f"P_{sg}_{jp}", bufs=2)
                    nc.scalar.activation(P, ps, Exp, scale=float(SCALE))
                    m = mask_pool.tile([128, 2, 128], BF16, tag="mask",
                                       name=f"m_{sg}_{jp}")
                    nc.sync.dma_start(
                        out=m,
                        in_=masks.rearrange("j p m -> p j m")[
                            :, MBASE[sg] + 2 * jp:MBASE[sg] + 2 * jp + 2, :],
                    )
                    nc.vector.tensor_mul(P, P, m)
                    for jj in range(2):
                        p_tiles[(sg, 2 * jp + jj)] = P[:, jj, :]

            def emit_ut(sg):
                """u^T[e, s-slot] = sum_j xrow[t,e-chunk]^T @ P_j (raw,
                unnormalized); den[s] via ones-moving matmul chain."""
                T = SLOT_T[sg]
                pu = pu_pool.tile([128, ET, 128], F32, tag="pu",
                                  name=f"pu_{sg}")
                pd = pd_pool.tile([128, 1], F32, tag="pd", name=f"pd_{sg}")
                for et in range(ET):
                    for j in range(T):
                        nc.tensor.matmul(
                            pu[:, et, :], xr[:, j, ts(et, 128)],
                            p_tiles[(sg, j)],
                            start=(j == 0), stop=(j == T - 1),
                        )
                for j in range(T):
                    nc.tensor.matmul(
                        pd, p_tiles[(sg, j)], ones_mv,
                        start=(j == 0), stop=(j == T - 1),
                    )
                dr = denr_pool.tile([128, 1], F32, tag="dr", name=f"dr_{sg}")
                nc.vector.reciprocal(dr, pd)
                ut = ut_pool.tile([128, ET, 128], BF16, tag="ut",
                                  name=f"ut_{sg}")
                for et in range(ET):
                    nc.vector.tensor_copy(ut[:, et, :], pu[:, et, :])
                ut_tiles[sg] = (ut, dr)

            def emit_final(sg):
                rows = 128 * sg
                ut, dr = ut_tiles[sg]
                for fb in range(2):
                    po = po_pool.tile([128, 512], F32, tag="po",
                                      name=f"po_{sg}_{fb}")
                    for et in range(ET):
                        nc.tensor.matmul(
                            po, ut[:, et, :], wv[:, et, ts(fb, 512)],
                            start=(et == 0), stop=(et == ET - 1),
                        )
                    osb = osb_pool.tile([128, 512], F32, tag="osb",
                                        name=f"osb_{sg}_{fb}")
                    # out = (uT @ wv) * (1/den)  (per-partition scale), + bv
                    nc.scalar.activation(osb, po, Ident, scale=dr)
                    nc.vector.tensor_add(osb, osb, bvb[:, ts(fb, 512)])
                    nc.sync.dma_start(
                        out=out_sd[rows:rows + 128, ts(fb, 512)], in_=osb)

            # Pipeline: scores(s_a) ahead of ut(s_a) ahead of final(s_a),
            # with the next slot's scores interleaved so PE never waits on
            # ACT/DVE.
            order = list(SLOT_ORDER)
            emit_scores(order[0])
            for idx, sg in enumerate(order):
                if idx + 1 < len(order):
                    emit_scores(order[idx + 1])
                emit_ut(sg)
                if idx >= 1:
                    emit_final(order[idx - 1])
            emit_final(order[-1])

    nc.compile()
    return nc


def _get_program():
    if "nc" not in _CACHE:
        _CACHE["nc"] = _build_program()
    return _CACHE["nc"]


def _perm_indices(h):
    return np.concatenate(
        [np.arange(128 * b, 128 * (b + 1)) for b in PERM_BLOCKS[h]])


def _host_prep(x, Wk, bk, Wq, bq, Wv, bv):
    """Build per-core in_maps (bf16 streams)."""
    import ml_dtypes
    f32 = np.float32
    bf16 = ml_dtypes.bfloat16

    Wk64 = np.asarray(Wk, np.float64)
    Wq64 = np.asarray(Wq, np.float64)
    M32 = np.ascontiguousarray((Wk64.T @ Wq64) * 32.0).astype(f32)
    zb = (np.asarray(Wq, f32).T @ np.asarray(bk, f32))  # [E] per-e' bias on z
    zbpv = np.ascontiguousarray(zb.reshape(ET, 128).T)
    wvTc = np.ascontiguousarray(np.asarray(Wv, f32).T).astype(bf16)
    mwc = M32.astype(bf16)
    bvc = np.ascontiguousarray(np.asarray(bv, f32))

    in_maps = []
    for c in range(NCORES):
        b, h = divmod(c, 2)
        perm = _perm_indices(h)
        xb = np.asarray(x[b], f32)
        xTb = np.ascontiguousarray(xb.T[:, perm]).astype(bf16)
        # local query columns in slot order: slot sg occupies cols
        # [128*sg, 128*sg+128) drawn from permuted position 256*sg.
        qcols = np.concatenate(
            [perm[SRC[sg]:SRC[sg] + 128] for sg in range(8)])
        xTqb = np.ascontiguousarray(xb.T[:, qcols]).astype(bf16)
        # xrow: [128, ST, XA] — x rows in permuted order + ones column.
        xrb = np.ones((S, XA), f32)
        xrb[:, :E] = xb[perm, :]
        xrowb = np.ascontiguousarray(
            xrb.reshape(ST, 128, XA).transpose(1, 0, 2)).astype(bf16)
        m = np.zeros((NMASK, 128, 128), f32)
        for sg in range(8):
            s_g = perm[SRC[sg]:SRC[sg] + 128]
            for j in range(SLOT_T[sg]):
                t_g = perm[128 * j:128 * (j + 1)]
                m[MBASE[sg] + j] = (t_g[:, None] <= s_g[None, :]).astype(f32)
        in_maps.append({
            "xT": xTb, "xTq": xTqb, "xrow": xrowb,
            "mw": mwc, "wvT": wvTc, "zbp": zbpv, "bv": bvc,
            "masks": m.astype(bf16),
        })
    return in_maps


def _assemble(results):
    out = np.empty((B, S, E), np.float32)
    for c in range(NCORES):
        b, h = divmod(c, 2)
        perm = _perm_indices(h)
        osd = results[c]["out_sd"]  # [1024, E], rows = slot-local queries
        for sg in range(8):
            rows = perm[SRC[sg]:SRC[sg] + 128]
            out[b, rows, :] = osd[128 * sg:128 * (sg + 1), :]
    return out


def kernel(x, Wk, bk, Wq, bq, Wv, bv):
    _ensure_concourse()
    from concourse.bass_utils import run_bass_kernel_spmd
    nc = _get_program()
    in_maps = _host_prep(x, Wk, bk, Wq, bq, Wv, bv)
    res = run_bass_kernel_spmd(nc, in_maps, list(range(NCORES)))
    return _assemble(res.results)


# revision 18
# speedup vs baseline: 1.8342x; 1.2418x over previous
"""Trainium2 Bass kernel for single-head causal attention (nn_DefaultAttention).

Reference computation (B=4, S=2048, E=1024, fp32):
    k = x @ Wk.T + bk ; q = x @ Wq.T + bq ; v = x @ Wv.T + bv
    sim[b,s,t] = k[b,s]·q[b,t] / sqrt(E), masked to t<=s
    out[b,s]   = softmax_t(sim[b,s,:]) @ v[b,:]

Algebraic folding (single head => the QK / V weight folds are exact):
    sim*sqrt(E) = x (Wk^T Wq) x^T + terms;  the per-s and const bias terms
    are constant along the softmax axis t and cancel; the per-t term
    (Wq^T bk)·x[t] folds into a per-feature bias c on z := x M
    (M = Wk^T Wq, host-computed).  So
        scores[s,t] = (x[s] M + c) · x[t]
    and the q/k projections never run on-device.  Likewise
        out = P @ v / den = (P @ x) @ Wv^T / den + bv
    so the v projection becomes a post-GEMM on P@x.  Per-core work drops
    from 2.5 projection-equivalents + attention to 1.0 + attention.

fp8 DoubleRow residual arithmetic: every operand of the z / scores / u^T
GEMMs is split into hi + lo fp8 parts (e4m3 for x/M/z, e5m2 for P whose
range exceeds e4m3), and products are computed with the 3-term expansion
hi*hi + hi*lo + lo*hi (lo*lo ~ eps^2 dropped).  DoubleRow packs 2
contraction tiles per instruction at 0.5 cycles/row, so the 3-term fp8
GEMM costs 0.75x of its bf16 version while being *more* accurate
(residual quantization error ~eps^2).  The final (u/den) @ Wv^T GEMM
stays bf16: its fp8 savings are small and u would need an extra
normalize+split round trip.

Sharding: 8 cores = 4 batches x 2 interleaved 128-row query-block sets.
Slot k=0..7 processes a uniform T=2k+2 key tiles against the query block
at permuted position 2k; per-core 0/1 masks zero invalid t>s entries
after exp (softmax's shift-invariance disposes of the per-s bias terms).

Dataflow per core (PSUM fp32 accumulation everywhere):
    z^T[e',s]  = sum_e M32[e,e'] x[s,e] / 32 + c[e']     (ACT bias+scale,
                 split into e4m3 hi/lo: ACT hi, ACT tmp, DVE lo)
    ps[t,s]    = sum_e x^T[e,t] z^T[e,s]                 (3-term fp8)
    Pb[t,s]    = exp(ps/sqrt(E)) * mask  (ACT+DVE) -> e5m2 hi/lo
    uT[e,s]    = sum_t xrow[t,e] P[t,s]  (3-term fp8), den via ones moving
    out[s,f]   = (sum_e uT[e,s] wvT[e,f]) * (1/den) + bv (bf16 GEMM; ACT
                 per-partition reciprocal scale; DVE bias add)
"""

import numpy as np


def _ensure_concourse():
    try:
        import concourse  # noqa: F401
    except ImportError:
        import sys
        for p in ("/opt/trn_rl_repo", "/root/.axon_site/_ro/trn_rl_repo"):
            if p not in sys.path:
                sys.path.append(p)
        import concourse  # noqa: F401


E = 1024
S = 2048
B = 4
NCORES = 8
ET = E // 128    # 8 feature tiles
ST = S // 128    # 16 key tiles
SCALE = 1.0 / np.sqrt(np.float32(E))
SLOT_T = (2, 4, 6, 8, 10, 12, 14, 16)   # key-128-tiles per slot (uniform)
SRC = tuple(256 * k for k in range(8))   # permuted query-col base per slot
MBASE = (0, 2, 6, 12, 20, 30, 42, 56)    # flat mask index base per slot
NMASK = 72
SLOT_ORDER = (7, 6, 5, 4, 3, 2, 1, 0)    # processing order (biggest T first)
# 128-row-block permutations: slot k (T=2k+2) hosts the core's query block
# with causal need n_k (A: {1,4,5,8,9,12,13,16}, B: {2,3,6,7,10,11,14,15})
# at position 2k; every block's needed key tiles land within [0, T_k).
PERM_BLOCKS = {
    0: [0, 1, 3, 2, 4, 5, 7, 6, 8, 9, 11, 10, 12, 13, 15, 14],
    1: [1, 0, 2, 3, 5, 4, 6, 7, 9, 8, 10, 11, 13, 12, 14, 15],
}
XA = 1025                      # xrow free width: 1024 features + ones col

_CACHE = {}
import os as _os
_PROBE = _os.environ.get("KERNEL_PROBE", "")  # '', 'z', 'zs', 'zsu'


def _build_program():
    _ensure_concourse()
    from contextlib import ExitStack
    import concourse.tile as tile
    import concourse.bass as bass
    from concourse import bacc, mybir

    F32 = mybir.dt.float32
    BF16 = mybir.dt.bfloat16
    F8 = mybir.dt.float8e4
    F8E5 = mybir.dt.float8e5
    DR = mybir.MatmulPerfMode.DoubleRow
    ts = bass.ts
    Exp = mybir.ActivationFunctionType.Exp
    Ident = mybir.ActivationFunctionType.Identity

    nc = bacc.Bacc("TRN2", target_bir_lowering=False, debug=False)

    xTh = nc.dram_tensor("xTh", [E, S], F8, kind="ExternalInput").ap()
    xTl = nc.dram_tensor("xTl", [E, S], F8, kind="ExternalInput").ap()
    xrowh = nc.dram_tensor("xrowh", [128, ST, XA], F8, kind="ExternalInput").ap()
    xrowl = nc.dram_tensor("xrowl", [128, ST, XA], F8, kind="ExternalInput").ap()
    mwh = nc.dram_tensor("mwh", [E, E], F8, kind="ExternalInput").ap()
    mwl = nc.dram_tensor("mwl", [E, E], F8, kind="ExternalInput").ap()
    wvT = nc.dram_tensor("wvT", [E, E], BF16, kind="ExternalInput").ap()
    zbp = nc.dram_tensor("zbp", [128, ET], F32, kind="ExternalInput").ap()
    bv = nc.dram_tensor("bv", [E], BF16, kind="ExternalInput").ap()
    # masks: [128, NMASK*128] p-major so each slot's block is contiguous
    # along the free dim (single descriptor per partition row).
    masks = nc.dram_tensor("masks", [128, NMASK * 128], F8, kind="ExternalInput").ap()
    out_sd = nc.dram_tensor("out_sd", [1024, E], BF16, kind="ExternalOutput").ap()

    with tile.TileContext(nc) as tc, ExitStack() as top:
        # ---- persistent smalls -------------------------------------------
        smalls = top.enter_context(tc.tile_pool(name="smalls", bufs=1))
        zb_sb = smalls.tile([128, ET], F32)
        nc.sync.dma_start(out=zb_sb, in_=zbp)
        bvb = smalls.tile([128, E], BF16)
        bv_bcast = bass.AP(tensor=bv.tensor, offset=bv.offset,
                           ap=[[0, 128]] + list(bv.ap))
        ones2 = smalls.tile([128, 2, 1], F8E5)
        nc.vector.memset(ones2, 1.0)

        # Warm the ACT function tables (Identity/Exp) up front so the
        # LoadActFuncSet DMA doesn't queue behind the bulk loads later.
        scratch = smalls.tile([1, 8], F32)
        nc.vector.memset(scratch, 0.0)
        nc.scalar.activation(scratch, scratch, Ident, bias=0.0, scale=1.0)
        nc.scalar.activation(scratch, scratch, Exp, scale=1.0)

        # ---- big persistent operands -------------------------------------
        big = top.enter_context(tc.tile_pool(name="big", bufs=1))
        xth = big.tile([128, ET, S], F8)        # x^T permuted hi
        xtl = big.tile([128, ET, S], F8)        # x^T permuted lo
        xrh = big.tile([128, ST, XA], F8)       # x row-major + ones, hi
        xrl = big.tile([128, ST, XA], F8)       # x row-major + zeros, lo
        wv = big.tile([128, ET, E], BF16)       # Wv^T as [e, f]
        zth = big.tile([128, ET, 1024], F8)     # z^T hi
        ztl = big.tile([128, ET, 1024], F8)     # z^T lo
        mk = big.tile([128, NMASK, 128], F8)    # all causal masks

        # mw hi/lo only live through phase Z; scoped pool.
        zin_ctx = tc.tile_pool(name="zin", bufs=1, side="right")
        zin = zin_ctx.__enter__()
        mwth = zin.tile([128, ET, E], F8)
        mwtl = zin.tile([128, ET, E], F8)

        def load_rearr(eng, dst, src, db, width=512):
            eng.dma_start(
                out=dst[:, :, ts(db, width)],
                in_=src.rearrange("(e p) s -> p e s", p=128)[:, :, ts(db, width)],
            )

        # All transfers serialize on the shared DMA-engines device
        # (~344 GB/s): what matters is total bytes and consumption order.
        # Everything rides the SP queue: ACT-queue DMAs would steal
        # Activation SEQ time from exp/Ident dispatch.
        load_rearr(nc.sync, xth, xTh, 3, 512)   # slots 6,7 first
        load_rearr(nc.sync, xtl, xTl, 3, 512)
        for q in range(4):                      # stream mw in quarters
            load_rearr(nc.sync, mwth, mwh, q, 256)
            load_rearr(nc.sync, mwtl, mwl, q, 256)
        load_rearr(nc.sync, xth, xTh, 2, 512)
        load_rearr(nc.sync, xtl, xTl, 2, 512)
        for cb in (0, 1):
            load_rearr(nc.sync, xth, xTh, cb, 512)
            load_rearr(nc.sync, xtl, xTl, cb, 512)
        nc.sync.dma_start(out=mk, in_=masks.rearrange("p (j m) -> p j m", m=128))
        for jc in range(4):
            nc.sync.dma_start(out=xrh[:, 4 * jc:4 * jc + 4, :],
                              in_=xrowh[:, 4 * jc:4 * jc + 4, :])
            nc.sync.dma_start(out=xrl[:, 4 * jc:4 * jc + 4, :],
                              in_=xrowl[:, 4 * jc:4 * jc + 4, :])
        load_rearr(nc.sync, wv, wvT, 0, 512)
        load_rearr(nc.sync, wv, wvT, 1, 512)
        nc.sync.dma_start(out=bvb, in_=bv_bcast)

        def mm3(out_ap, stat_h, stat_l, mov_h, mov_l, npair, first, last):
            """3-term fp8 residual product into one PSUM accumulation
            chain: hh, hl, lh — each as DoubleRow pairs over npair
            contraction-tile pairs.  stat_*/mov_* map pair index ->
            [128, 2, N] APs."""
            seq = [(stat_h, mov_h), (stat_h, mov_l), (stat_l, mov_h)]
            n = 3 * npair
            i = 0
            for sh, mv in seq:
                for p in range(npair):
                    nc.tensor.matmul(
                        out_ap, sh(p), mv(p),
                        start=(first and i == 0),
                        stop=(last and i == n - 1),
                        perf_mode=DR,
                    )
                    i += 1

        # ---- phase Z: z^T = (x M32)/32 + c, local queries, split hi/lo ---
        proj_ctx = tc.tile_pool(name="pz", bufs=4, space="PSUM")
        proj_pool = proj_ctx.__enter__()
        ztm_ctx = tc.tile_pool(name="ztm", bufs=3)
        ztm_pool = ztm_ctx.__enter__()
        for spair in ((6, 7), (4, 5), (0, 1), (2, 3)):
            for ep in range(ET):
                pz = proj_pool.tile([128, 2, 128], F32, tag="pz")
                for i, sg in enumerate(spair):
                    mm3(
                        pz[:, i, :],
                        lambda p: mwth[:, 2 * p:2 * p + 2, ts(ep, 128)],
                        lambda p: mwtl[:, 2 * p:2 * p + 2, ts(ep, 128)],
                        lambda p, sg=sg: xth[:, 2 * p:2 * p + 2,
                                             SRC[sg]:SRC[sg] + 128],
                        lambda p, sg=sg: xtl[:, 2 * p:2 * p + 2,
                                             SRC[sg]:SRC[sg] + 128],
                        npair=ET // 2, first=True, last=True,
                    )
                c0 = 128 * spair[0]
                nc.scalar.activation(zth[:, ep, c0:c0 + 256], pz, Ident,
                                     bias=zb_sb[:, ep:ep + 1],
                                     scale=float(1.0 / 32.0))
                ztm = ztm_pool.tile([128, 2, 128], BF16, tag="ztm")
                nc.scalar.activation(ztm, pz, Ident,
                                     bias=zb_sb[:, ep:ep + 1],
                                     scale=float(1.0 / 32.0))
                nc.vector.tensor_sub(ztl[:, ep, c0:c0 + 256], ztm,
                                     zth[:, ep, c0:c0 + 256])
        ztm_ctx.__exit__(None, None, None)
        proj_ctx.__exit__(None, None, None)
        zin_ctx.__exit__(None, None, None)

        # ---- fused scores -> P -> u^T -> final GEMM, per slot ------------
        # PSUM budget (8 banks): ps pairs 3 + pu 2 + pd 1 + po 2.
        p_pool = top.enter_context(tc.tile_pool(name="pP", bufs=1))
        p_hi = {}
        p_lo = {}

        with tc.tile_pool(name="osb", bufs=3) as osb_pool, \
             tc.tile_pool(name="ut", bufs=2) as ut_pool, \
             tc.tile_pool(name="denr", bufs=2) as denr_pool, \
             tc.tile_pool(name="pb", bufs=3) as pb_pool, \
             tc.tile_pool(name="ps", bufs=3, space="PSUM") as ps_pool, \
             tc.tile_pool(name="pu", bufs=1, space="PSUM") as pu_pool, \
             tc.tile_pool(name="pd", bufs=1, space="PSUM") as pd_pool, \
             tc.tile_pool(name="po", bufs=2, space="PSUM") as po_pool:

            ut_tiles = {}

            def emit_scores(sg):
                # j-pairs share one PSUM bank; chains are emitted
                # sequentially so the pending-zero region flip is safe.
                T = SLOT_T[sg]
                for jp in range(T // 2):
                    ps = ps_pool.tile([128, 2, 128], F32, tag="ps",
                                      name=f"ps_{sg}_{jp}")
                    for jj in range(2):
                        j = 2 * jp + jj
                        mm3(
                            ps[:, jj, :],
                            lambda p, j=j: xth[:, 2 * p:2 * p + 2, ts(j, 128)],
                            lambda p, j=j: xtl[:, 2 * p:2 * p + 2, ts(j, 128)],
                            lambda p: zth[:, 2 * p:2 * p + 2, ts(sg, 128)],
                            lambda p: ztl[:, 2 * p:2 * p + 2, ts(sg, 128)],
                            npair=ET // 2, first=True, last=True,
                        )
                    Pb = pb_pool.tile([128, 2, 128], BF16, tag="pb",
                                      name=f"pb_{sg}_{jp}")
                    nc.scalar.activation(Pb, ps, Exp, scale=float(SCALE))
                    nc.vector.tensor_mul(Pb, Pb,
                                         mk[:, MBASE[sg] + 2 * jp:
                                            MBASE[sg] + 2 * jp + 2, :])
                    Ph = p_pool.tile([128, 2, 128], F8E5, tag=f"Ph{jp}",
                                     name=f"Ph_{sg}_{jp}", bufs=2)
                    nc.scalar.copy(Ph, Pb)
                    Pl = p_pool.tile([128, 2, 128], F8E5, tag=f"Pl{jp}",
                                     name=f"Pl_{sg}_{jp}", bufs=2)
                    nc.vector.tensor_sub(Pl, Pb, Ph)
                    p_hi[(sg, jp)] = Ph
                    p_lo[(sg, jp)] = Pl

            def emit_ut(sg):
                """u^T[e, s-slot] = sum_j xrow[t,e-chunk]^T @ P_j (raw,
                unnormalized); den[s] via ones-moving matmul chain."""
                T = SLOT_T[sg]
                npair = T // 2
                pu = pu_pool.tile([128, ET, 128], F32, tag="pu",
                                  name=f"pu_{sg}")
                pd = pd_pool.tile([128, 1], F32, tag="pd", name=f"pd_{sg}")
                for et in range(ET):
                    mm3(
                        pu[:, et, :],
                        lambda p, et=et: xrh[:, 2 * p:2 * p + 2, ts(et, 128)],
                        lambda p, et=et: xrl[:, 2 * p:2 * p + 2, ts(et, 128)],
                        lambda p: p_hi[(sg, p)],
                        lambda p: p_lo[(sg, p)],
                        npair=npair, first=True, last=True,
                    )
                for jp in range(npair):
                    nc.tensor.matmul(pd, p_hi[(sg, jp)], ones2,
                                     start=(jp == 0), stop=False,
                                     perf_mode=DR)
                for jp in range(npair):
                    nc.tensor.matmul(pd, p_lo[(sg, jp)], ones2,
                                     start=False, stop=(jp == npair - 1),
                                     perf_mode=DR)
                dr = denr_pool.tile([128, 1], F32, tag="dr", name=f"dr_{sg}")
                nc.vector.reciprocal(dr, pd)
                ut = ut_pool.tile([128, ET, 128], BF16, tag="ut",
                                  name=f"ut_{sg}")
                for et in range(ET):
                    nc.vector.tensor_copy(ut[:, et, :], pu[:, et, :])
                ut_tiles[sg] = (ut, dr)

            def emit_final(sg):
                rows = 128 * sg
                ut, dr = ut_tiles[sg]
                for fb in range(2):
                    po = po_pool.tile([128, 512], F32, tag="po",
                                      name=f"po_{sg}_{fb}")
                    for et in range(ET):
                        nc.tensor.matmul(
                            po, ut[:, et, :], wv[:, et, ts(fb, 512)],
                            start=(et == 0), stop=(et == ET - 1),
                        )
                    osb = osb_pool.tile([128, 512], BF16, tag="osb",
                                        name=f"osb_{sg}_{fb}")
                    # out = (uT @ wv) * (1/den)  (per-partition scale), + bv
                    nc.scalar.activation(osb, po, Ident, scale=dr)
                    nc.vector.tensor_add(osb, osb, bvb[:, ts(fb, 512)])
                    nc.sync.dma_start(
                        out=out_sd[rows:rows + 128, ts(fb, 512)], in_=osb)

            # Pipeline: scores(s_a) ahead of ut(s_a) ahead of final(s_a),
            # with the next slot's scores interleaved so PE never waits on
            # ACT/DVE.
            order = list(SLOT_ORDER)
            if _PROBE == "z":
                order = []
            if order:
                emit_scores(order[0])
            for idx, sg in enumerate(order):
                if idx + 1 < len(order):
                    emit_scores(order[idx + 1])
                if _PROBE == "zs":
                    continue
                emit_ut(sg)
                if _PROBE == "zsu":
                    continue
                if idx >= 1:
                    emit_final(order[idx - 1])
            if order and _PROBE == "":
                emit_final(order[-1])

    nc.compile()
    return nc


def _get_program():
    if "nc" not in _CACHE:
        _CACHE["nc"] = _build_program()
    return _CACHE["nc"]


def _perm_indices(h):
    return np.concatenate(
        [np.arange(128 * b, 128 * (b + 1)) for b in PERM_BLOCKS[h]])


def _split8(a, dt):
    hi = np.asarray(a, dtype=dt)
    lo = np.asarray(np.asarray(a, np.float32) - hi.astype(np.float32),
                    dtype=dt)
    return hi, lo


def _host_prep(x, Wk, bk, Wq, bq, Wv, bv):
    """Build per-core in_maps (fp8 hi/lo + bf16 streams)."""
    import ml_dtypes
    f32 = np.float32
    bf16 = ml_dtypes.bfloat16
    f8 = ml_dtypes.float8_e4m3

    Wk64 = np.asarray(Wk, np.float64)
    Wq64 = np.asarray(Wq, np.float64)
    M32 = np.ascontiguousarray((Wk64.T @ Wq64) * 32.0).astype(f32)
    mwhc, mwlc = _split8(M32, f8)
    zb = (np.asarray(Wq, f32).T @ np.asarray(bk, f32))  # [E] per-e' bias on z
    zbpv = np.ascontiguousarray(zb.reshape(ET, 128).T)
    wvTc = np.ascontiguousarray(np.asarray(Wv, f32).T).astype(bf16)
    bvc = np.ascontiguousarray(np.asarray(bv, f32)).astype(bf16)

    in_maps = []
    for c in range(NCORES):
        b, h = divmod(c, 2)
        perm = _perm_indices(h)
        xb = np.asarray(x[b], f32)
        xTh_, xTl_ = _split8(np.ascontiguousarray(xb.T[:, perm]), f8)
        # xrow: [128, ST, XA] — x rows in permuted order + ones column.
        xrb = np.ones((S, XA), f32)
        xrb[:, :E] = xb[perm, :]
        xrowb = np.ascontiguousarray(
            xrb.reshape(ST, 128, XA).transpose(1, 0, 2))
        xrh_, xrl_ = _split8(xrowb, f8)
        m = np.zeros((NMASK, 128, 128), f32)
        for sg in range(8):
            s_g = perm[SRC[sg]:SRC[sg] + 128]
            for j in range(SLOT_T[sg]):
                t_g = perm[128 * j:128 * (j + 1)]
                m[MBASE[sg] + j] = (t_g[:, None] <= s_g[None, :]).astype(f32)
        mp = np.ascontiguousarray(
            m.transpose(1, 0, 2).reshape(128, NMASK * 128))
        in_maps.append({
            "xTh": xTh_, "xTl": xTl_, "xrowh": xrh_, "xrowl": xrl_,
            "mwh": mwhc, "mwl": mwlc, "wvT": wvTc, "zbp": zbpv, "bv": bvc,
            "masks": mp.astype(f8),
        })
    return in_maps


def _assemble(results):
    out = np.empty((B, S, E), np.float32)
    for c in range(NCORES):
        b, h = divmod(c, 2)
        perm = _perm_indices(h)
        osd = np.asarray(results[c]["out_sd"], np.float32)
        for sg in range(8):
            rows = perm[SRC[sg]:SRC[sg] + 128]
            out[b, rows, :] = osd[128 * sg:128 * (sg + 1), :]
    return out


def kernel(x, Wk, bk, Wq, bq, Wv, bv):
    _ensure_concourse()
    from concourse.bass_utils import run_bass_kernel_spmd
    nc = _get_program()
    in_maps = _host_prep(x, Wk, bk, Wq, bq, Wv, bv)
    res = run_bass_kernel_spmd(nc, in_maps, list(range(NCORES)))
    return _assemble(res.results)
